# revision 14
# baseline (speedup 1.0000x reference)
"""Trainium2 Bass kernel for the binarized CNN (BNN) forward pass.

Network (reference semantics, f32):
  h1 = conv2d(x, sign(w1))            # [B,8,26,26]
  p1 = maxpool2(sign(h1))             # [B,8,13,13]  (sign commutes with max)
  h2 = conv2d(p1, sign(w2))           # [B,16,11,11] (p1 already +-1)
  p2 = maxpool2(h2)                   # [B,16,5,5]
  out = sign(p2.reshape(B,400)) @ sign(wfc).T   # [B,10]

Strategy: pure data parallel over 8 NeuronCores (2048 images each).

conv1 is a Toeplitz-over-height matmul: K=(dj,iin)=84, M=(oc,pooled-row)=104.
The output-row parity (iout = 2*i + par) is folded into the WEIGHTS: two
parity passes write two 1-bank PSUM tiles, so both 2x2-pool axes end up in
the free dimension (vertical = parity-tile max, horizontal = jout-parity
strided max) — SBUF tensor_tensor ops then always see equal base
partitions, which the walrus verifier requires.  x is fed as an fp16 hi +
fp16 lo pair for ~f32 accuracy; weights are exact +-1 so all later stages
are exact integer arithmetic in fp16/f32.

conv2: K=(cin,row13)=104 matching conv1's pooled layout, 3 dj passes x 2
parity tiles.  fc: lhsT = pooled signs (batch in M) so the result lands
batch-major [128, 10] and needs no transpose.

Each PSUM tile's first matmul is a 1x1 "wait-absorber" dummy (reads only
the resident weight tile): it soaks up the slot-recycle WAR wait so no
real matmul ever carries more than one semaphore wait — the TRN2 Matmult
instruction struct only has room for one (walrus "Too many sync wait
commands" otherwise).
"""

import numpy as np
from contextlib import ExitStack

import concourse.bass as bass
import concourse.tile as tile
from concourse import bacc, mybir
from concourse.bass_utils import run_bass_kernel_spmd

F16 = mybir.dt.float16
F32 = mybir.dt.float32

NCORES = 8
B = 16384
BC = B // NCORES          # images per core
NB = 256                  # images per outer batch (DMA granularity)
NB_COUNT = BC // NB       # 8
NB1 = 16                  # conv1 images per psum tile (N = 26*16 = 416)
NB2 = 32                  # conv2 images per psum tile (N = 10*32 = 320)
NB3 = 128                 # fc images per psum tile  (M = 128, N = 10)

HMAX1_ENGINE = "vector"   # engine for pool1 horizontal max


def _build_weights(w1, w2, wfc):
    """Host-side: binarize + lay out the Toeplitz / fc weight matrices."""
    w1b = np.sign(w1).astype(np.float32)    # [8,1,3,3]
    w2b = np.sign(w2).astype(np.float32)    # [16,8,3,3]
    wfb = np.sign(wfc).astype(np.float32)   # [10,400]

    # conv1: lhsT [K=84=(dj,iin), 208=(par | oc8, i13)], iout = 2*i + par
    w1t = np.zeros((84, 208), np.float32)
    for par in range(2):
        for oc in range(8):
            for i in range(13):
                col = par * 104 + oc * 13 + i
                for dj in range(3):
                    for di in range(3):
                        w1t[dj * 28 + 2 * i + par + di, col] = w1b[oc, 0, di, dj]

    # conv2: lhsT [K=104=(cin,row13), 480=(pass=(dj,par) | oc16, i5)],
    # iout2 = 2*i + par, input row = iout2 + di
    w2t = np.zeros((104, 480), np.float32)
    for dj in range(3):
        for par in range(2):
            p = dj * 2 + par
            for oc2 in range(16):
                for i in range(5):
                    col = p * 80 + oc2 * 5 + i
                    for cin in range(8):
                        for di in range(3):
                            w2t[cin * 13 + 2 * i + par + di, col] = (
                                w2b[oc2, cin, di, dj]
                            )

    # fc: rhs [K=80=(oc2,i2p), 50=(j2p, cls)]
    wft = np.zeros((80, 50), np.float32)
    for oc2 in range(16):
        for i2p in range(5):
            row = oc2 * 5 + i2p
            for j in range(5):
                for cls in range(10):
                    wft[row, j * 10 + cls] = wfb[cls, oc2 * 25 + i2p * 5 + j]

    return (w1t.astype(np.float16), w2t.astype(np.float16), wft.astype(np.float16))


def _build_xr(x):
    """Host-side: fp16 hi/lo split + dj-replicated, height-partition layout.

    Returns [NCORES, 2, 84, BC, 26] fp16 where
      xr[c, s, dj*28+iin, b, jin] = split_s(x[c*BC + b, 0, iin, jin + dj])
    (one DMA per batch covers all 84 K-partitions and both splits at once)
    """
    x4 = np.ascontiguousarray(x.reshape(B, 28, 28), dtype=np.float32)
    hi = x4.astype(np.float16)
    lo = (x4 - hi.astype(np.float32)).astype(np.float16)
    xr = np.empty((NCORES, 2, 3, 28, BC, 26), np.float16)
    for s, arr in enumerate((hi, lo)):
        a = arr.reshape(NCORES, BC, 28, 28)
        for dj in range(3):
            # [c, b, iin, jin] -> [c, iin, b, jin]
            xr[:, s, dj] = a[:, :, :, dj:dj + 26].transpose(0, 2, 1, 3)
    return xr.reshape(NCORES, 2, 84, BC, 26)


def _build_program():
    nc = bacc.Bacc(
        "TRN2",
        target_bir_lowering=False,
        debug=False,
        enable_asserts=False,
        num_devices=NCORES,
    )
    xr_d = nc.dram_tensor("xr", [2, 84, BC, 26], F16, kind="ExternalInput").ap()
    w1_d = nc.dram_tensor("w1t", [84, 208], F16, kind="ExternalInput").ap()
    w2_d = nc.dram_tensor("w2t", [104, 480], F16, kind="ExternalInput").ap()
    wf_d = nc.dram_tensor("wfct", [80, 50], F16, kind="ExternalInput").ap()
    out_d = nc.dram_tensor("out", [BC, 10], F32, kind="ExternalOutput").ap()

    hmax1 = getattr(nc, HMAX1_ENGINE)

    with tile.TileContext(nc) as tc, ExitStack() as ctx:
        wpool = ctx.enter_context(tc.tile_pool(name="weights", bufs=1))
        xrp = ctx.enter_context(tc.tile_pool(name="xr", bufs=2))
        s1p = ctx.enter_context(tc.tile_pool(name="s1", bufs=4))
        v1p = ctx.enter_context(tc.tile_pool(name="v1", bufs=3))
        h1pp = ctx.enter_context(tc.tile_pool(name="h1p", bufs=2))
        s2pl = ctx.enter_context(tc.tile_pool(name="s2", bufs=4))
        v2p = ctx.enter_context(tc.tile_pool(name="v2", bufs=3))
        s2pp = ctx.enter_context(tc.tile_pool(name="s2p", bufs=2))
        oap = ctx.enter_context(tc.tile_pool(name="outacc", bufs=1))
        c1ps = ctx.enter_context(tc.tile_pool(name="c1ps", bufs=3, space="PSUM"))
        c2ps = ctx.enter_context(tc.tile_pool(name="c2ps", bufs=3, space="PSUM"))
        fcps = ctx.enter_context(tc.tile_pool(name="fcps", bufs=2, space="PSUM"))

        w1_sb = wpool.tile([84, 208], F16)
        nc.sync.dma_start(w1_sb[:], w1_d)
        w2_sb = wpool.tile([104, 480], F16)
        nc.sync.dma_start(w2_sb[:], w2_d)
        wf_sb = wpool.tile([80, 50], F16)
        nc.sync.dma_start(wf_sb[:], wf_d)

        outacc = oap.tile([128, NB_COUNT * 2 * 10], F32)

        for t in range(NB_COUNT):
            # ---- one DMA loads the whole batch (both splits, all 84 rows)
            xr_t = xrp.tile([84, 2, NB * 26], F16)
            nc.sync.dma_start(
                xr_t[:],
                xr_d[:, :, t * NB:(t + 1) * NB, :].rearrange(
                    "s p b j -> p s (b j)"
                ),
            )

            # ---- conv1 + pool1 -> h1p [104=(cin,row13), NB*13]
            h1p = h1pp.tile([104, NB * 13], F16)
            for c in range(NB // NB1):
                sps = []
                for par in range(2):
                    ps = c1ps.tile([104, 512], F32, tag="c1")
                    # wait-absorber: soaks the slot-recycle WAR wait
                    nc.tensor.matmul(
                        ps[0:1, 448:449], w1_sb[:, 0:1], w1_sb[:, 0:1],
                        start=True, stop=True,
                    )
                    for s in range(2):
                        rhs = (
                            xr_t[:, s, :]
                            .rearrange("p (b j) -> p b j", j=26)[
                                :, c * NB1:(c + 1) * NB1, :
                            ]
                            .rearrange("p b (k jp) -> p b jp k", jp=2)
                        )
                        nc.tensor.matmul(
                            ps[:, 0:416],
                            w1_sb[:, par * 104:(par + 1) * 104],
                            rhs,
                            start=(s == 0),
                            stop=(s == 1),
                        )
                    s1 = s1p.tile([104, 416], F16, tag="s1")
                    nc.scalar.sign(s1[:], ps[:, 0:416])
                    sps.append(s1)
                v1 = v1p.tile([104, 416], F16)
                nc.vector.tensor_max(v1[:], sps[0][:], sps[1][:])
                va = v1[:].rearrange("p (b jp k) -> p b jp k", jp=2, k=13)
                dst = h1p[
                    :, c * NB1 * 13:(c + 1) * NB1 * 13
                ].rearrange("p (b k) -> p b k", k=13)
                hmax1.tensor_max(dst, va[:, :, 0, :], va[:, :, 1, :])

            # ---- conv2 + pool2 + fc
            h1r = h1p[:].rearrange("p (b j) -> p b j", j=13)
            for c2 in range(NB // NB2):
                if c2 % 4 == 0:
                    s2p = s2pp.tile([80, NB3 * 5], F16)
                s2s = []
                for par in range(2):
                    ps2 = c2ps.tile([104, 512], F32, tag="c2")
                    nc.tensor.matmul(
                        ps2[0:1, 448:449], w2_sb[:, 0:1], w2_sb[:, 0:1],
                        start=True, stop=True,
                    )
                    for dj in range(3):
                        rhs = h1r[
                            :, c2 * NB2:(c2 + 1) * NB2, dj:dj + 10
                        ].rearrange("p b (k jp) -> p b jp k", jp=2)
                        nc.tensor.matmul(
                            ps2[0:80, 0:320],
                            w2_sb[:, (dj * 2 + par) * 80:(dj * 2 + par + 1) * 80],
                            rhs,
                            start=(dj == 0),
                            stop=(dj == 2),
                        )
                    s2 = s2pl.tile([80, 320], F16, tag="s2")
                    nc.scalar.sign(s2[:], ps2[0:80, 0:320])
                    s2s.append(s2)
                v2 = v2p.tile([80, 320], F16)
                nc.vector.tensor_max(v2[:], s2s[0][:], s2s[1][:])
                vb = v2[:].rearrange("p (b jp k) -> p b jp k", jp=2, k=5)
                dst = s2p[
                    :, (c2 % 4) * NB2 * 5:((c2 % 4) + 1) * NB2 * 5
                ].rearrange("p (b k) -> p b k", k=5)
                nc.vector.tensor_max(dst, vb[:, :, 0, :], vb[:, :, 1, :])

                if c2 % 4 == 3:
                    fidx = t * 2 + c2 // 4
                    psf = fcps.tile([128, 512], F32, tag="fc")
                    nc.tensor.matmul(
                        psf[0:1, 448:449], wf_sb[:, 0:1], wf_sb[:, 0:1],
                        start=True, stop=True,
                    )
                    s2r = s2p[:].rearrange("p (b j) -> p j b", j=5)
                    for j in range(5):
                        nc.tensor.matmul(
                            psf[:, 0:10],
                            s2r[:, j, :],
                            wf_sb[:, j * 10:(j + 1) * 10],
                            start=(j == 0),
                            stop=(j == 4),
                        )
                    nc.scalar.copy(outacc[:, fidx * 10:(fidx + 1) * 10], psf[:, 0:10])

        nc.sync.dma_start(
            out_d.rearrange("(t p) c -> p t c", p=128),
            outacc[:].rearrange("p (t c) -> p t c", c=10),
        )

    nc.compile()
    return nc


_NC_CACHE = {}


def kernel(x, w1, w2, wfc, _trace=False):
    x = np.asarray(x, dtype=np.float32)
    w1 = np.asarray(w1, dtype=np.float32)
    w2 = np.asarray(w2, dtype=np.float32)
    wfc = np.asarray(wfc, dtype=np.float32)

    w1t, w2t, wft = _build_weights(w1, w2, wfc)
    xr = _build_xr(x)

    if "nc" not in _NC_CACHE:
        _NC_CACHE["nc"] = _build_program()
    nc = _NC_CACHE["nc"]

    in_maps = [
        {
            "xr": np.ascontiguousarray(xr[c]),
            "w1t": w1t,
            "w2t": w2t,
            "wfct": wft,
        }
        for c in range(NCORES)
    ]
    res = run_bass_kernel_spmd(
        nc, in_maps, core_ids=list(range(NCORES)), trace=_trace
    )
    _NC_CACHE["last_res"] = res
    out = np.concatenate(
        [res.results[c]["out"] for c in range(NCORES)], axis=0
    ).astype(np.float32)
    return out
